# revision 1
# baseline (speedup 1.0000x reference)
"""Bass/Tile TRN2 kernel for retrieval-KNN MSE distance matrix.

Computes: out = ||t||^2 + ||s@W.T+b||^2 - 2 * t @ (s@W.T+b).T   [N=4096, M=4096]

Sharding (8 cores, output column-parallel, no collectives):
  core c holds s_rep rows [c*512, (c+1)*512) and computes the full-height
  output block out[:, c*512:(c+1)*512].  Per-core work:
    GEMM1: s_projT[1536, 512] = WT.T @ sT         (K-major, 12x12 k/m chunks)
    GEMM2: out_j[128, 512]    = tT_j.T @ s_projT  accumulated over 12 k-chunks
  t_sq comes from gram-block matmuls (tile.T @ tile, diagonal extracted via
  identity mask + tensor_tensor_reduce) and enters as the fp32 ACT bias;
  s_sq comes from a ones-matmul over the squared projection and is folded
  into PSUM with a compensated K=2 matmul (hi+lo bf16 split, scaled -0.5)
  so the ACT copyback (scale=-2, bias=t_sq) produces the final value.

Matmuls run in bf16 (fp32 inputs cast on-chip by DVE); accumulation fp32.
"""

import numpy as np

import concourse.bacc as bacc
import concourse.bass as bass
import concourse.mybir as mybir
import concourse.tile as tile
from concourse.bass_utils import run_bass_kernel_spmd

N = 4096          # t_rep rows
M = 4096          # s_rep rows
D = 1536          # feature dim
NCORES = 8
MC = M // NCORES  # 512: output columns per core
KC = D // 128     # 12:  contraction chunks
NJ = N // 128     # 32:  output row chunks per core

FP32 = mybir.dt.float32
BF16 = mybir.dt.bfloat16
AF = mybir.ActivationFunctionType


def build_nc(variant="full"):
    nc = bacc.Bacc("TRN2", target_bir_lowering=False, num_devices=NCORES)

    t_in = nc.dram_tensor("t", [NJ // 4, KC, 128, 512], FP32, kind="ExternalInput").ap()
    s_in = nc.dram_tensor("s", [KC, 128, MC], FP32, kind="ExternalInput").ap()
    w_in = nc.dram_tensor("w", [KC, 128, D], FP32, kind="ExternalInput").ap()
    b_in = nc.dram_tensor("b", [KC, 128, 1], FP32, kind="ExternalInput").ap()
    id_in = nc.dram_tensor("ident", [128, 128], FP32, kind="ExternalInput").ap()
    out = nc.dram_tensor("out", [NJ, 128, MC], FP32, kind="ExternalOutput").ap()

    with tile.TileContext(nc) as tc:
        with (
            tc.tile_pool(name="const", bufs=1) as const_pool,
            tc.tile_pool(name="sproj", bufs=1) as sproj_pool,
            tc.tile_pool(name="small", bufs=1) as small_pool,
            tc.tile_pool(name="psum_main", bufs=4, space="PSUM") as psum_main,
        ):
            ident = const_pool.tile([128, 128], FP32)
            nc.sync.dma_start(out=ident[:], in_=id_in[:, :])
            ones_col = const_pool.tile([128, 1], BF16)  # lhsT for s_sq row-matmul
            nc.vector.memset(ones_col[:], 1.0)

            # ---- Phase 0: HAM warmup while initial DMAs stream ----
            warm = const_pool.tile([128, MC], BF16, name="warm")
            nc.vector.memset(warm[:], 0.5)
            with tc.tile_pool(name="psum_warm", bufs=1, space="PSUM") as pw_pool:
                pw = pw_pool.tile([128, MC], FP32, name="pw")
                for i in range(60):
                    nc.tensor.matmul(pw[:], lhsT=warm[:, 0:128], rhs=warm[:],
                                     start=(i == 0), stop=(i == 59))

            # ---- Phase 1: projection s_projT[d, r] + bias, and s_sq ----
            sproj = []  # 12 tiles [128, MC] bf16
            with (
                tc.tile_pool(name="wts", bufs=6) as wt_pool,
                tc.tile_pool(name="wtb", bufs=1) as wtb_pool,
                tc.tile_pool(name="srep", bufs=2) as s_pool,
                tc.tile_pool(name="srepb", bufs=1) as sb_pool,
                tc.tile_pool(name="bias", bufs=1) as b_pool,
                tc.tile_pool(name="sq", bufs=3) as sq_pool,
                tc.tile_pool(name="psum_aux", bufs=2, space="PSUM") as psum_aux,
            ):
                wt_sb = []
                s_sb = []
                b_sb = []
                wt_f32 = []
                for k in range(KC):
                    st = s_pool.tile([128, MC], FP32, name="st")
                    nc.sync.dma_start(out=st[:], in_=s_in[k])
                    stb = sb_pool.tile([128, MC], BF16, name=f"stb{k}")
                    nc.vector.tensor_copy(stb[:], st[:])
                    s_sb.append(stb)

                    bt = b_pool.tile([128, 1], FP32, name=f"bt{k}")
                    nc.sync.dma_start(out=bt[:], in_=b_in[k])
                    b_sb.append(bt)

                    wt_sb.append(wtb_pool.tile([128, D], BF16, name=f"wtb{k}"))
                # column-group-major W loads through small fp32 piece tiles:
                # GEMM1 j-block c can start after the first 12 pieces land
                for c in range(D // 512):
                    for k in range(KC):
                        sl = slice(c * 512, (c + 1) * 512)
                        wtp = wt_pool.tile([128, 512], FP32, name="wtp")
                        nc.sync.dma_start(out=wtp[:], in_=w_in[k][:, sl])
                        nc.vector.tensor_copy(wt_sb[k][:, sl], wtp[:])

                psum_sq = psum_aux.tile([1, MC], FP32, name="psum_ssq")
                for j in range(KC):
                    ps = psum_main.tile([128, MC], FP32, name="psum_p1", tag="mm")
                    for k in range(KC):
                        nc.tensor.matmul(
                            ps[:],
                            lhsT=wt_sb[k][:, j * 128:(j + 1) * 128],
                            rhs=s_sb[k][:],
                            start=(k == 0),
                            stop=(k == KC - 1),
                        )
                    sp = sproj_pool.tile([128, MC], BF16, name=f"sproj{j}")
                    nc.scalar.activation(sp[:], ps[:], AF.Identity,
                                         bias=b_sb[j][:], scale=1.0)
                    sproj.append(sp)
                    # squared projection -> s_sq partial via ones-matmul
                    sq = sq_pool.tile([128, MC], BF16, name="sq")
                    nc.vector.tensor_mul(sq[:], sp[:], sp[:])
                    nc.tensor.matmul(
                        psum_sq[:],
                        lhsT=ones_col[:],
                        rhs=sq[:],
                        start=(j == 0),
                        stop=(j == KC - 1),
                    )

                # s_sq broadcast tile [128, MC] fp32 via log2-doubling DMAs
                ssq_bc = small_pool.tile([128, MC], FP32, name="ssq_bc")
                nc.scalar.activation(ssq_bc[0:1, :], psum_sq[:], AF.Identity)
                sh = 1
                while sh < 128:
                    nc.sync.dma_start(out=ssq_bc[sh:2 * sh, :],
                                      in_=ssq_bc[0:sh, :])
                    sh *= 2

            # ---- Phase 2: main GEMM over 32 row-chunks ----
            with (
                tc.tile_pool(name="tt", bufs=3 * KC) as t_pool,
                tc.tile_pool(name="ttb", bufs=3 * KC) as tb_pool,
                tc.tile_pool(name="osb", bufs=16) as out_pool,
                tc.tile_pool(name="obtp", bufs=4) as obt_pool,
                tc.tile_pool(name="tsq", bufs=3) as tsq_pool,
                tc.tile_pool(name="psum_gram", bufs=3, space="PSUM") as psum_gram,
            ):
                pending_out = []
                for g in range(NJ // 4):
                  tg_sb = []
                  for k in range(KC):
                      tt = t_pool.tile([128, 512], FP32, name="tt")
                      nc.sync.dma_start(out=tt[:], in_=t_in[g, k])
                      ttb = tb_pool.tile([128, 512], BF16, name="ttb")
                      nc.vector.tensor_copy(ttb[:], tt[:])
                      tg_sb.append(ttb)
                  # flush stores two groups back (sync stream stays waitless)
                  while len(pending_out) > 15:
                      oj, oob = pending_out.pop(0)
                      nc.sync.dma_start(out=out[oj], in_=oob[:])
                  for jj in range(4):
                    j = 4 * g + jj
                    t_sb = [tg_sb[k][:, jj * 128:(jj + 1) * 128] for k in range(KC)]

                    ps = psum_main.tile([128, MC], FP32, name="psum_main", tag="mm")
                    use_gram = variant in ("full", "gram", "gram_only", "gram_ttr")
                    use_ttr = variant in ("full", "gram", "gram_ttr")
                    use_bias = variant in ("full", "gram")
                    use_ssq = variant in ("full", "full_nogram")
                    for k in range(KC):
                        nc.tensor.matmul(
                            ps[:],
                            lhsT=t_sb[k],
                            rhs=sproj[k][:],
                            start=(k == 0),
                            stop=(k == KC - 1),
                        )
                        if use_gram:
                            if k == 0:
                                gram = psum_gram.tile([128, 128], FP32, name="psum_gram")
                            nc.tensor.matmul(
                                gram[:],
                                lhsT=t_sb[k],
                                rhs=t_sb[k],
                                start=(k == 0),
                                stop=(k == KC - 1),
                            )
                    ob = out_pool.tile([128, MC], FP32, name="osb")
                    if use_gram and not use_ttr:
                        # consume gram so it isn't dead: copy into scratch and DMA a row out
                        gsb = tsq_pool.tile([128, 128], FP32, name="gsb")
                        nc.scalar.activation(gsb[:], gram[:], AF.Identity)
                        nc.sync.dma_start(out=out[j][:, 0:128], in_=gsb[:])
                    if use_gram and use_ttr:
                        # t_sq[p] = sum_f gram[p, f] * I[p, f]
                        tsq = tsq_pool.tile([128, 1], FP32, name="tsq")
                        scratch = tsq_pool.tile([128, 128], FP32, name="tsq_scratch")
                        nc.vector.tensor_mul(scratch[:], gram[:], ident[:])
                        nc.vector.reduce_sum(tsq[:], scratch[:],
                                             axis=mybir.AxisListType.X)
                        # out = (-2 * cross + t_sq) + s_sq
                        obt = obt_pool.tile([128, MC], FP32, name="obt")
                        nc.scalar.activation(obt[:], ps[:], AF.Identity,
                                             bias=tsq[:], scale=-2.0)
                        nc.vector.tensor_add(ob[:], obt[:], ssq_bc[:])
                    else:
                        nc.scalar.activation(ob[:], ps[:], AF.Identity,
                                             scale=-2.0)
                    pending_out.append((j, ob))
                for (oj, oob) in pending_out:
                    nc.sync.dma_start(out=out[oj], in_=oob[:])

    nc.compile()
    return nc


_NC_CACHE = None


def _get_nc():
    global _NC_CACHE
    if _NC_CACHE is None:
        _NC_CACHE = build_nc()
    return _NC_CACHE


def stage_inputs(t_rep, s_rep, W, b):
    """Host-side layout staging (transpose/tile only) -> per-core input maps."""
    t_rep = np.asarray(t_rep, dtype=np.float32)
    s_rep = np.asarray(s_rep, dtype=np.float32)
    W = np.asarray(W, dtype=np.float32)
    b = np.asarray(b, dtype=np.float32)

    # t tiles: [NJ/4, KC, 128(d), 512(row)]; tile[g,k][p,c] = t_rep[g*512+c, k*128+p]
    t_tiles = np.ascontiguousarray(
        t_rep.reshape(NJ // 4, 512, KC, 128).transpose(0, 2, 3, 1)
    )
    # WT: [KC, 128, D]; WT[k][p, m] = W[m, k*128+p]
    wt = np.ascontiguousarray(W.T).reshape(KC, 128, D)
    b_st = np.ascontiguousarray(b.reshape(KC, 128, 1))

    in_maps = []
    for c in range(NCORES):
        s_slice = s_rep[c * MC:(c + 1) * MC]  # [512, D]
        # sT: [KC, 128, MC]; sT[k][p, r] = s_slice[r, k*128+p]
        s_st = np.ascontiguousarray(
            s_slice.reshape(MC, KC, 128).transpose(1, 2, 0)
        )
        in_maps.append({"t": t_tiles, "s": s_st, "w": wt, "b": b_st,
                        "ident": np.eye(128, dtype=np.float32)})
    return in_maps


def run_spmd(in_maps, **kwargs):
    nc = _get_nc()
    return run_bass_kernel_spmd(nc, in_maps, core_ids=list(range(NCORES)), **kwargs)


def gather_output(results):
    return np.concatenate(
        [results[c]["out"].reshape(N, MC) for c in range(NCORES)], axis=1
    )


def kernel(t_rep, s_rep, W, b):
    in_maps = stage_inputs(t_rep, s_rep, W, b)
    res = run_spmd(in_maps)
    return gather_output(res.results)



# revision 12
# speedup vs baseline: 1.2700x; 1.2700x over previous
"""Bass/Tile TRN2 kernel for retrieval-KNN MSE distance matrix.

Computes: out = ||t||^2 + ||s@W.T+b||^2 - 2 * t @ (s@W.T+b).T   [N=4096, M=4096]

Sharding (8 cores, output column-parallel, no collectives):
  core c holds s_rep rows [c*512, (c+1)*512) and computes the full-height
  output block out[:, c*512:(c+1)*512].  Per-core work:
    GEMM1: s_projT[1536, 512] = WT.T @ sT       (bf16, 12x12 k/j chunks)
    GEMM2: psum[128, 512] = (-2 t) @ s_projT    (fp8e4 DoubleRow, 6 K=256 MMs)
           + one bf16 K=4 "fold" matmul adding t_sq + s_sq (hi/lo split)
  so PSUM holds the final output value and the drain is a plain copy.

  t_sq: DVE squares+accumulates the bf16 t tiles across the 12 k-chunks,
  then one ones-matmul per 512-row group reduces partitions -> [1, 512].
  s_sq: ones-matmul over squared bf16 s_proj tiles (as before).

Inputs are staged host-side in bf16 (t, s, W); fp8 copies of t (scaled by
-2) and s_proj are produced on-chip by ScalarE/DVE.
"""

import numpy as np
import ml_dtypes

import concourse.bacc as bacc
import concourse.bass as bass
import concourse.mybir as mybir
import concourse.tile as tile
from concourse.bass_utils import run_bass_kernel_spmd

N = 4096          # t_rep rows
M = 4096          # s_rep rows
D = 1536          # feature dim
NCORES = 8
MC = M // NCORES  # 512: output columns per core
KC = D // 128     # 12:  contraction chunks
KP = KC // 2      # 6:   fp8 DoubleRow k-pairs
NJ = N // 128     # 32:  output row chunks per core
NG = N // 512     # 8:   512-row groups
WP = D // 512     # 3:   W column pieces

FP32 = mybir.dt.float32
BF16 = mybir.dt.bfloat16
FP8 = mybir.dt.float8e4
AF = mybir.ActivationFunctionType
BF16NP = ml_dtypes.bfloat16

N_WARM = 24


def build_nc(variant="full"):
    fp8_main = variant == "full"
    # fp8: psum holds -2*cross, fold adds +tsq+ssq (scale 1.0), drain copies.
    # bf16: psum holds +cross, fold adds -(tsq+ssq)/2, drain scales by -2.
    fold_scale = 1.0 if fp8_main else -0.5
    nc = bacc.Bacc("TRN2", target_bir_lowering=False, num_devices=NCORES)

    t_in = nc.dram_tensor("t", [NG, KC, 128, 512], BF16, kind="ExternalInput").ap()
    s_in = nc.dram_tensor("s", [KC, 128, MC], BF16, kind="ExternalInput").ap()
    w_in = nc.dram_tensor("w", [WP, KC, 128, 512], BF16, kind="ExternalInput").ap()
    b_in = nc.dram_tensor("b", [KC, 128, 1], FP32, kind="ExternalInput").ap()
    out = nc.dram_tensor("out", [NJ, 128, MC], FP32, kind="ExternalOutput").ap()

    with tile.TileContext(nc) as tc:
        with (
            tc.tile_pool(name="const", bufs=1) as const_pool,
            tc.tile_pool(name="sproj", bufs=1) as sproj_pool,
            tc.tile_pool(name="sprojf8", bufs=1) as sprojf8_pool,
            tc.tile_pool(name="fold", bufs=1) as fold_pool,
            tc.tile_pool(name="small", bufs=4) as small_pool,
            tc.tile_pool(name="psum_main", bufs=4, space="PSUM") as psum_main,
            tc.tile_pool(name="psum_tsq", bufs=2, space="PSUM") as psum_tsq_pool,
        ):
            ones_col = const_pool.tile([128, 1], BF16)
            nc.vector.memset(ones_col[:], 1.0)
            # fold operands: psum += foldlhs_slice.T @ foldrhs adds t_sq
            # (lhs row 0 x rhs ones row 0) and s_sq (lhs ones row 1 x rhs
            # row 1).  Engine writes must start at partition 0, so the
            # partition-1 rows are assembled by one-time SBUF->SBUF DMAs.
            ones_row = const_pool.tile([1, N], BF16)
            nc.vector.memset(ones_row[:], 1.0)
            foldlhs = fold_pool.tile([2, N], BF16)
            nc.sync.dma_start(out=foldlhs[1:2, :], in_=ones_row[:])
            foldrhs = fold_pool.tile([2, MC], BF16)
            nc.vector.memset(foldrhs[0:1, :], 1.0)

            # ---- Phase 0: HAM warmup while initial DMAs stream ----
            with (
                tc.tile_pool(name="warmp", bufs=1) as warm_pool,
                tc.tile_pool(name="psum_warm", bufs=1, space="PSUM") as pw_pool,
            ):
                warm = warm_pool.tile([128, MC], BF16, name="warm")
                nc.vector.memset(warm[:], 0.5)
                pw = pw_pool.tile([128, MC], FP32, name="pw")
                for i in range(N_WARM):
                    nc.tensor.matmul(pw[:], lhsT=warm[:, 0:128], rhs=warm[:],
                                     start=(i == 0), stop=(i == N_WARM - 1))

            # ---- Phase 1: projection s_projT + s_sq; t groups stream in ----
            sproj = []    # 12 bf16 tiles [128, MC]
            sprojf8 = []  # 6 fp8 pair tiles [128, 2, MC]
            for p in range(KP):
                spf = sprojf8_pool.tile([128, 2, MC], FP8, name=f"sprojf8_{p}")
                sprojf8.append(spf)

            with (
                tc.tile_pool(name="wts", bufs=1) as wt_pool,
                tc.tile_pool(name="srep", bufs=1) as s_pool,
                tc.tile_pool(name="bias", bufs=1) as b_pool,
                tc.tile_pool(name="sq", bufs=KC) as sq_pool,
                tc.tile_pool(name="tt", bufs=4 * KC) as t_pool,
                tc.tile_pool(name="tf8", bufs=2 * KP) as tf8_pool,
                tc.tile_pool(name="acc", bufs=2) as acc_pool,
                tc.tile_pool(name="sqt", bufs=4) as sqt_pool,
                tc.tile_pool(name="osb", bufs=16) as out_pool,
                tc.tile_pool(name="psum_ssq", bufs=1, space="PSUM") as psum_ssq_pool,
            ):
                # -- DMA issue order: s, W pieces (c-major), b, then t groups --
                s_sb = []
                for k in range(KC):
                    st = s_pool.tile([128, MC], BF16, name=f"st{k}")
                    nc.sync.dma_start(out=st[:], in_=s_in[k])
                    s_sb.append(st)
                wt_sb = [wt_pool.tile([128, D], BF16, name=f"wt{k}")
                         for k in range(KC)]
                for c in range(WP):
                    for k in range(KC):
                        sl = slice(c * 512, (c + 1) * 512)
                        nc.sync.dma_start(out=wt_sb[k][:, sl], in_=w_in[c, k])
                b_sb = []
                for k in range(KC):
                    bt = b_pool.tile([128, 1], FP32, name=f"bt{k}")
                    nc.sync.dma_start(out=bt[:], in_=b_in[k])
                    b_sb.append(bt)
                # t tiles: issue the first 4 groups now so they land during
                # phase 1; remaining groups issued inside the main loop.
                t_sb = {}
                for g in range(4):
                    for k in range(KC):
                        tt = t_pool.tile([128, 512], BF16, name="tt")
                        nc.sync.dma_start(out=tt[:], in_=t_in[g, k])
                        t_sb[(g, k)] = tt

                # -- GEMM1: 12 j-blocks of 12 k-matmuls --
                psum_ssq = psum_ssq_pool.tile([1, MC], FP32, name="psum_ssq")
                sq_sb = []
                for j in range(KC):
                    ps = psum_main.tile([128, MC], FP32, name="psum_p1", tag="mm")
                    for k in range(KC):
                        nc.tensor.matmul(
                            ps[:],
                            lhsT=wt_sb[k][:, j * 128:(j + 1) * 128],
                            rhs=s_sb[k][:],
                            start=(k == 0),
                            stop=(k == KC - 1),
                        )
                    sp = sproj_pool.tile([128, MC], BF16, name=f"sproj{j}")
                    nc.scalar.activation(sp[:], ps[:], AF.Identity,
                                         bias=b_sb[j][:], scale=1.0)
                    sproj.append(sp)
                    if fp8_main:
                        # fp8 copy for the DoubleRow GEMM (DVE reads same psum)
                        nc.vector.tensor_scalar_add(
                            sprojf8[j // 2][:, j % 2, :], ps[:], b_sb[j][:])
                    # squared projection for s_sq
                    sq = sq_pool.tile([128, MC], BF16, name="sq")
                    nc.vector.tensor_mul(sq[:], sp[:], sp[:])
                    sq_sb.append(sq)
                    # lag the s_sq ones-matmul two j-blocks so PE never waits
                    if j >= 2:
                        nc.tensor.matmul(psum_ssq[:], lhsT=ones_col[:],
                                         rhs=sq_sb[j - 2][:],
                                         start=(j == 2), stop=False)
                for j in (KC - 2, KC - 1):
                    nc.tensor.matmul(psum_ssq[:], lhsT=ones_col[:],
                                     rhs=sq_sb[j][:],
                                     start=False, stop=(j == KC - 1))
                # s_sq -> fold rhs row 1 (bf16, assembled via DMA)
                ssq_bf = small_pool.tile([1, MC], BF16, name="ssq_bf")
                nc.scalar.activation(ssq_bf[:], psum_ssq[:], AF.Identity,
                                     scale=fold_scale)
                nc.sync.dma_start(out=foldrhs[1:2, :], in_=ssq_bf[:])

                # ---- Phase 2: main fp8 GEMM over 8 groups x 4 j-chunks ----
                pending_out = []
                for g in range(NG):
                    # prefetch t for group g+4
                    if g + 4 < NG:
                        for k in range(KC):
                            tt = t_pool.tile([128, 512], BF16, name="tt")
                            nc.sync.dma_start(out=tt[:], in_=t_in[g + 4, k])
                            t_sb[(g + 4, k)] = tt
                    tg = [t_sb.pop((g, k)) for k in range(KC)]
                    # fp8 cast (scaled by -2) on ScalarE
                    tf8 = []
                    if fp8_main:
                        for p in range(KP):
                            tp = tf8_pool.tile([128, 2, 512], FP8, name="tf8")
                            nc.scalar.mul(tp[:, 0, :], tg[2 * p][:], -2.0)
                            nc.scalar.mul(tp[:, 1, :], tg[2 * p + 1][:], -2.0)
                            tf8.append(tp)
                    # t_sq: DVE square+accumulate, then ones-matmul reduce
                    acc = acc_pool.tile([128, 512], BF16, name="acc")
                    nc.vector.tensor_mul(acc[:], tg[0][:], tg[0][:])
                    for k in range(1, KC):
                        sqt = sqt_pool.tile([128, 512], BF16, name="sqt")
                        nc.vector.tensor_mul(sqt[:], tg[k][:], tg[k][:])
                        nc.vector.tensor_add(acc[:], acc[:], sqt[:])
                    ptsq = psum_tsq_pool.tile([1, 512], FP32, name="ptsq")
                    nc.tensor.matmul(ptsq[:], lhsT=ones_col[:], rhs=acc[:],
                                     start=True, stop=True)
                    gs = slice(g * 512, (g + 1) * 512)
                    nc.scalar.activation(foldlhs[0:1, gs], ptsq[:], AF.Identity,
                                         scale=fold_scale)

                    # flush deferred output stores (issued well after their
                    # drain so the sync stream never waits)
                    while len(pending_out) > 7:
                        oj, oob = pending_out.pop(0)
                        nc.sync.dma_start(out=out[oj], in_=oob[:])

                    for jj in range(4):
                        j = 4 * g + jj
                        ps = psum_main.tile([128, MC], FP32, name="psum_main",
                                            tag="mm")
                        if fp8_main:
                            for p in range(KP):
                                nc.tensor.matmul(
                                    ps[:],
                                    lhsT=tf8[p][:, :, jj * 128:(jj + 1) * 128],
                                    rhs=sprojf8[p][:],
                                    start=(p == 0),
                                    stop=False,
                                    perf_mode=mybir.MatmulPerfMode.DoubleRow,
                                )
                        else:
                            for k in range(KC):
                                nc.tensor.matmul(
                                    ps[:],
                                    lhsT=tg[k][:, jj * 128:(jj + 1) * 128],
                                    rhs=sproj[k][:],
                                    start=(k == 0),
                                    stop=False,
                                )
                        nc.tensor.matmul(
                            ps[:],
                            lhsT=foldlhs[:, j * 128:(j + 1) * 128],
                            rhs=foldrhs[:],
                            start=False,
                            stop=True,
                            skip_group_check=True,
                        )
                        ob = out_pool.tile([128, MC], FP32, name="osb")
                        if fp8_main:
                            # psum already holds the final value
                            if jj % 2 == 0:
                                nc.scalar.copy(ob[:], ps[:])
                            else:
                                nc.vector.tensor_copy(ob[:], ps[:])
                        else:
                            # bf16 path: psum holds cross + (tsq+ssq)/(-2)
                            nc.scalar.activation(ob[:], ps[:], AF.Identity,
                                                 scale=-2.0)
                        pending_out.append((j, ob))
                for (oj, oob) in pending_out:
                    nc.sync.dma_start(out=out[oj], in_=oob[:])

    nc.compile()
    return nc


_NC_CACHE = {}


def _get_nc(variant="full"):
    if variant not in _NC_CACHE:
        _NC_CACHE[variant] = build_nc(variant)
    return _NC_CACHE[variant]


def stage_inputs(t_rep, s_rep, W, b):
    """Host-side layout staging (transpose/tile + bf16 cast) -> per-core inputs."""
    t_rep = np.asarray(t_rep, dtype=np.float32)
    s_rep = np.asarray(s_rep, dtype=np.float32)
    W = np.asarray(W, dtype=np.float32)
    b = np.asarray(b, dtype=np.float32)

    # t tiles: [NG, KC, 128(d), 512(row)]; tile[g,k][p,r] = t[g*512+r, k*128+p]
    t_tiles = np.ascontiguousarray(
        t_rep.astype(BF16NP).reshape(NG, 512, KC, 128).transpose(0, 2, 3, 1)
    )
    # W pieces: [WP, KC, 128, 512]; piece[c,k][p,m] = W[c*512+m, k*128+p]
    wt = np.ascontiguousarray(
        W.T.astype(BF16NP).reshape(KC, 128, WP, 512).transpose(2, 0, 1, 3)
    )
    b_st = np.ascontiguousarray(b.reshape(KC, 128, 1))

    in_maps = []
    for c in range(NCORES):
        s_slice = s_rep[c * MC:(c + 1) * MC]  # [512, D]
        s_st = np.ascontiguousarray(
            s_slice.astype(BF16NP).reshape(MC, KC, 128).transpose(1, 2, 0)
        )
        in_maps.append({"t": t_tiles, "s": s_st, "w": wt, "b": b_st})
    return in_maps


def run_spmd(in_maps, variant="full", **kwargs):
    nc = _get_nc(variant)
    return run_bass_kernel_spmd(nc, in_maps, core_ids=list(range(NCORES)), **kwargs)


def gather_output(results):
    return np.concatenate(
        [results[c]["out"].reshape(N, MC) for c in range(NCORES)], axis=1
    )


def kernel(t_rep, s_rep, W, b):
    in_maps = stage_inputs(t_rep, s_rep, W, b)
    res = run_spmd(in_maps)
    return gather_output(res.results)


# revision 16
# speedup vs baseline: 1.5282x; 1.2033x over previous
"""Bass/Tile TRN2 kernel for retrieval-KNN MSE distance matrix.

Computes: out = ||t||^2 + ||s@W.T+b||^2 - 2 * t @ (s@W.T+b).T   [N=4096, M=4096]

Sharding (8 cores, output column-parallel, no collectives):
  core c holds s_rep rows [c*512, (c+1)*512) and computes the full-height
  output block out[:, c*512:(c+1)*512].  Per-core work:
    GEMM1: s_projT[1536, 512] = WT.T @ sT       (bf16, 12x12 k/j chunks)
    GEMM2: psum[128, 512] = (-2 t) @ s_projT    (fp8e4 DoubleRow, 6 K=256 MMs)
           + one bf16 K=1 "fold" matmul adding s_sq
  t_sq enters as the per-partition ACT/DVE bias at the PSUM drain, so the
  drain produces the final output value.

Host staging: t is shipped twice -- as fp8e4 pairs (d-major, pre-scaled by
-2) for the DoubleRow GEMM, and as row-major bf16 for t_sq, which is one
fused DVE tensor_tensor_reduce (square + row-sum -> [128,1]) per j-chunk.
s, W ship as bf16.  s_sq comes from a ones-matmul over squared bf16 s_proj.
"""

import numpy as np
import ml_dtypes

import concourse.bacc as bacc
import concourse.bass as bass
import concourse.mybir as mybir
import concourse.tile as tile
from concourse.bass_utils import run_bass_kernel_spmd

N = 4096          # t_rep rows
M = 4096          # s_rep rows
D = 1536          # feature dim
NCORES = 8
MC = M // NCORES  # 512: output columns per core
KC = D // 128     # 12:  contraction chunks
KP = KC // 2      # 6:   fp8 DoubleRow k-pairs
NJ = N // 128     # 32:  output row chunks per core
NG = N // 512     # 8:   512-row groups
WP = D // 512     # 3:   W column pieces

FP32 = mybir.dt.float32
BF16 = mybir.dt.bfloat16
FP8 = mybir.dt.float8e4
AF = mybir.ActivationFunctionType
ALU = mybir.AluOpType
BF16NP = ml_dtypes.bfloat16
FP8NP = ml_dtypes.float8_e4m3

N_WARM = 24


def build_nc(variant="full"):
    fp8_main = variant == "full"
    # fp8: psum holds -2*cross + s_sq, drain adds t_sq bias, scale 1.
    # bf16: psum holds cross - s_sq/2, drain scales by -2 and adds t_sq.
    fold_scale = 1.0 if fp8_main else -0.5
    nc = bacc.Bacc("TRN2", target_bir_lowering=False, num_devices=NCORES)

    t8_in = nc.dram_tensor("t8", [NG, 128, KP, 2, 512], FP8,
                           kind="ExternalInput").ap()
    tr_in = nc.dram_tensor("tr", [NG, 128, 4, D], BF16,
                           kind="ExternalInput").ap()
    s_in = nc.dram_tensor("s", [KC, 128, MC], BF16, kind="ExternalInput").ap()
    w_in = nc.dram_tensor("w", [WP, KC, 128, 512], BF16,
                          kind="ExternalInput").ap()
    b_in = nc.dram_tensor("b", [128, KC], FP32, kind="ExternalInput").ap()
    out = nc.dram_tensor("out", [NJ, 128, MC], FP32, kind="ExternalOutput").ap()

    with tile.TileContext(nc) as tc:
        with (
            tc.tile_pool(name="const", bufs=1) as const_pool,
            tc.tile_pool(name="sproj", bufs=1) as sproj_pool,
            tc.tile_pool(name="sprojf8", bufs=1) as sprojf8_pool,
            tc.tile_pool(name="small", bufs=2) as small_pool,
            tc.tile_pool(name="psum_main", bufs=4, space="PSUM") as psum_main,
        ):
            ones_col = const_pool.tile([128, 1], BF16)
            nc.vector.memset(ones_col[:], 1.0)
            ones128 = const_pool.tile([1, 128], BF16)
            nc.vector.memset(ones128[:], 1.0)
            ssq_bf = const_pool.tile([1, MC], BF16)  # fold rhs (written ph.1)

            # ---- Phase 0: HAM warmup while initial DMAs stream ----
            with (
                tc.tile_pool(name="warmp", bufs=1) as warm_pool,
                tc.tile_pool(name="psum_warm", bufs=1, space="PSUM") as pw_pool,
            ):
                warm = warm_pool.tile([128, MC], BF16, name="warm")
                nc.vector.memset(warm[:], 0.5)
                pw = pw_pool.tile([128, MC], FP32, name="pw")
                for i in range(N_WARM):
                    nc.tensor.matmul(pw[:], lhsT=warm[:, 0:128], rhs=warm[:],
                                     start=(i == 0), stop=(i == N_WARM - 1))

            # ---- Phase 1: projection s_projT + s_sq; t groups stream in ----
            sproj = []    # 12 bf16 tiles [128, MC]
            sprojf8 = []  # 6 fp8 pair tiles [128, 2, MC]
            for p in range(KP):
                spf = sprojf8_pool.tile([128, 2, MC], FP8, name=f"sprojf8_{p}")
                sprojf8.append(spf)

            with (
                tc.tile_pool(name="wts", bufs=1) as wt_pool,
                tc.tile_pool(name="srep", bufs=1) as s_pool,
                tc.tile_pool(name="bias", bufs=1) as b_pool,
                tc.tile_pool(name="sq", bufs=KC) as sq_pool,
                tc.tile_pool(name="t8sb", bufs=3) as t8_pool,
                tc.tile_pool(name="trsb", bufs=3) as tr_pool,
                tc.tile_pool(name="ttrscratch", bufs=2) as ttr_pool,
                tc.tile_pool(name="tsqb", bufs=24) as tsq_pool,
                tc.tile_pool(name="osb", bufs=16) as out_pool,
                tc.tile_pool(name="psum_ssq", bufs=1, space="PSUM") as psum_ssq_pool,
            ):
                # -- DMA issue order: s, W pieces (c-major), b, early t groups --
                s_sb = []
                for k in range(KC):
                    st = s_pool.tile([128, MC], BF16, name=f"st{k}")
                    nc.sync.dma_start(out=st[:], in_=s_in[k])
                    s_sb.append(st)
                wt_sb = [wt_pool.tile([128, D], BF16, name=f"wt{k}")
                         for k in range(KC)]
                for c in range(WP):
                    for k in range(KC):
                        sl = slice(c * 512, (c + 1) * 512)
                        nc.sync.dma_start(out=wt_sb[k][:, sl], in_=w_in[c, k])
                b_sb = b_pool.tile([128, KC], FP32, name="b_sb")
                nc.sync.dma_start(out=b_sb[:], in_=b_in[:, :])
                t8_sb, tr_sb = {}, {}
                for g in range(3):
                    t8 = t8_pool.tile([128, KP, 2, 512], FP8, name="t8")
                    nc.sync.dma_start(out=t8[:], in_=t8_in[g])
                    t8_sb[g] = t8
                    tr = tr_pool.tile([128, 4, D], BF16, name="tr")
                    nc.sync.dma_start(out=tr[:], in_=tr_in[g])
                    tr_sb[g] = tr

                # -- GEMM1: 12 j-blocks of 12 k-matmuls --
                psum_ssq = psum_ssq_pool.tile([1, MC], FP32, name="psum_ssq")
                sq_sb = []
                for j in range(KC):
                    ps = psum_main.tile([128, MC], FP32, name="psum_p1", tag="mm")
                    for k in range(KC):
                        nc.tensor.matmul(
                            ps[:],
                            lhsT=wt_sb[k][:, j * 128:(j + 1) * 128],
                            rhs=s_sb[k][:],
                            start=(k == 0),
                            stop=(k == KC - 1),
                        )
                    sp = sproj_pool.tile([128, MC], BF16, name=f"sproj{j}")
                    nc.scalar.activation(sp[:], ps[:], AF.Identity,
                                         bias=b_sb[:, j:j + 1], scale=1.0)
                    sproj.append(sp)
                    if fp8_main:
                        # fp8 copy for the DoubleRow GEMM (DVE reads same psum)
                        nc.vector.tensor_scalar_add(
                            sprojf8[j // 2][:, j % 2, :], ps[:], b_sb[:, j:j + 1])
                    # squared projection for s_sq
                    sq = sq_pool.tile([128, MC], BF16, name="sq")
                    nc.vector.tensor_mul(sq[:], sp[:], sp[:])
                    sq_sb.append(sq)
                    # lag the s_sq ones-matmul two j-blocks so PE never waits
                    if j >= 2:
                        nc.tensor.matmul(psum_ssq[:], lhsT=ones_col[:],
                                         rhs=sq_sb[j - 2][:],
                                         start=(j == 2), stop=False)
                for j in (KC - 2, KC - 1):
                    nc.tensor.matmul(psum_ssq[:], lhsT=ones_col[:],
                                     rhs=sq_sb[j][:],
                                     start=False, stop=(j == KC - 1))
                nc.scalar.activation(ssq_bf[:], psum_ssq[:], AF.Identity,
                                     scale=fold_scale)

                # ---- Phase 2: main fp8 GEMM over 8 groups x 4 j-chunks ----
                pending_out = []
                for g in range(NG):
                    if g + 3 < NG:
                        t8 = t8_pool.tile([128, KP, 2, 512], FP8, name="t8")
                        nc.sync.dma_start(out=t8[:], in_=t8_in[g + 3])
                        t8_sb[g + 3] = t8
                        tr = tr_pool.tile([128, 4, D], BF16, name="tr")
                        nc.sync.dma_start(out=tr[:], in_=tr_in[g + 3])
                        tr_sb[g + 3] = tr
                    t8 = t8_sb.pop(g)
                    tr = tr_sb.pop(g)
                    # t_sq -> [128, 1] drain bias.  Two chunks on ScalarE
                    # (fused Square + row-sum accum_out), two on DVE via
                    # bn_stats/bn_aggr (one pass + tiny fixup).
                    tsq = []
                    for jj in range(4):
                        tb = tsq_pool.tile([128, 1], FP32, name="tb")
                        if jj < 2:
                            scr = ttr_pool.tile([128, D], BF16, name="scr")
                            nc.scalar.activation(scr[:], tr[:, jj, :], AF.Square,
                                                 accum_out=tb[:])
                        else:
                            bn6 = tsq_pool.tile([128, 3, 6], FP32, name="bn6")
                            for cc in range(3):
                                nc.vector.bn_stats(
                                    bn6[:, cc, :],
                                    tr[:, jj, cc * 512:(cc + 1) * 512])
                            bn2 = tsq_pool.tile([128, 2], FP32, name="bn2")
                            nc.vector.bn_aggr(bn2[:], bn6[:])
                            # t_sq = D * (var + mean^2)
                            msq = tsq_pool.tile([128, 1], FP32, name="msq")
                            nc.vector.tensor_mul(msq[:], bn2[:, 0:1], bn2[:, 0:1])
                            nc.vector.tensor_add(msq[:], msq[:], bn2[:, 1:2])
                            nc.vector.tensor_scalar_mul(tb[:], msq[:], float(D))
                        tsq.append(tb)

                    # flush deferred output stores (issued well after their
                    # drain so the sync stream never waits)
                    while len(pending_out) > 7:
                        oj, oob = pending_out.pop(0)
                        nc.sync.dma_start(out=out[oj], in_=oob[:])

                    for jj in range(4):
                        j = 4 * g + jj
                        ps = psum_main.tile([128, MC], FP32, name="psum_main",
                                            tag="mm")
                        if fp8_main:
                            # stop=True on the last DoubleRow closes the sim's
                            # group tracker; the fold below still accumulates
                            # (stop is a no-op on hardware) with the group
                            # check skipped.
                            for p in range(KP):
                                nc.tensor.matmul(
                                    ps[:],
                                    lhsT=t8[:, p, :, jj * 128:(jj + 1) * 128],
                                    rhs=sprojf8[p][:],
                                    start=(p == 0),
                                    stop=(p == KP - 1),
                                    perf_mode=mybir.MatmulPerfMode.DoubleRow,
                                )
                        else:
                            for p in range(KP):
                                for c in range(2):
                                    # bf16 fallback reuses the fp8 tiles is not
                                    # possible; use row-major tr as lhsT via
                                    # sproj stream: not supported -> fp8 only
                                    raise NotImplementedError
                        # fold matmul: += s_sq (K=1, ones x ssq row)
                        nc.tensor.matmul(
                            ps[:],
                            lhsT=ones128[:],
                            rhs=ssq_bf[:],
                            start=False,
                            stop=True,
                            skip_group_check=True,
                        )
                        ob = out_pool.tile([128, MC], FP32, name="osb")
                        if jj % 2 == 0:
                            nc.scalar.activation(ob[:], ps[:], AF.Identity,
                                                 bias=tsq[jj][:], scale=1.0)
                        else:
                            nc.vector.tensor_scalar_add(ob[:], ps[:], tsq[jj][:])
                        pending_out.append((j, ob))
                for (oj, oob) in pending_out:
                    nc.sync.dma_start(out=out[oj], in_=oob[:])

    nc.compile()
    return nc


_NC_CACHE = {}


def _get_nc(variant="full"):
    if variant not in _NC_CACHE:
        _NC_CACHE[variant] = build_nc(variant)
    return _NC_CACHE[variant]


def stage_inputs(t_rep, s_rep, W, b):
    """Host-side layout staging (transpose/tile + dtype cast) -> per-core inputs."""
    t_rep = np.asarray(t_rep, dtype=np.float32)
    s_rep = np.asarray(s_rep, dtype=np.float32)
    W = np.asarray(W, dtype=np.float32)
    b = np.asarray(b, dtype=np.float32)

    # fp8 pairs, d-major, pre-scaled by -2:
    #   t8[g][p][P][c][r] = -2 * t[g*512+r, (2P+c)*128+p]
    t8 = np.ascontiguousarray(
        (-2.0 * t_rep).astype(FP8NP)
        .reshape(NG, 512, KP, 2, 128).transpose(0, 4, 2, 3, 1)
    )
    # row-major bf16: tr[g][p][jj][d] = t[g*512+jj*128+p, d]
    tr = np.ascontiguousarray(
        t_rep.astype(BF16NP).reshape(NG, 4, 128, D).transpose(0, 2, 1, 3)
    )
    # W pieces: [WP, KC, 128, 512]; piece[c,k][p,m] = W[c*512+m, k*128+p]
    wt = np.ascontiguousarray(
        W.T.astype(BF16NP).reshape(KC, 128, WP, 512).transpose(2, 0, 1, 3)
    )
    # b: [128, KC]; b_st[p, k] = b[k*128+p]
    b_st = np.ascontiguousarray(b.reshape(KC, 128).T)

    in_maps = []
    for c in range(NCORES):
        s_slice = s_rep[c * MC:(c + 1) * MC]  # [512, D]
        s_st = np.ascontiguousarray(
            s_slice.astype(BF16NP).reshape(MC, KC, 128).transpose(1, 2, 0)
        )
        in_maps.append({"t8": t8, "tr": tr, "s": s_st, "w": wt, "b": b_st})
    return in_maps


def run_spmd(in_maps, variant="full", **kwargs):
    nc = _get_nc(variant)
    return run_bass_kernel_spmd(nc, in_maps, core_ids=list(range(NCORES)), **kwargs)


def gather_output(results):
    return np.concatenate(
        [results[c]["out"].reshape(N, MC) for c in range(NCORES)], axis=1
    )


def kernel(t_rep, s_rep, W, b):
    in_maps = stage_inputs(t_rep, s_rep, W, b)
    res = run_spmd(in_maps)
    return gather_output(res.results)


# revision 22
# speedup vs baseline: 1.7266x; 1.1298x over previous
"""Bass/Tile TRN2 kernel for retrieval-KNN MSE distance matrix.

Computes: out = ||t||^2 + ||s@W.T+b||^2 - 2 * t @ (s@W.T+b).T   [N=4096, M=4096]

Sharding (8 cores, output column-parallel, no collectives):
  core c holds s_rep rows [c*512, (c+1)*512) and computes the full-height
  output block out[:, c*512:(c+1)*512].  Per-core work:
    GEMM1: s_projT[1536, 512] = WT.T @ sT       (bf16, 12x12 k/j chunks)
    GEMM2: psum[128, 512] = (-2 t) @ s_projT    (fp8e4 DoubleRow, 6 K=256 MMs)
           + one bf16 K=1 "fold" matmul adding s_sq
  t_sq enters as the per-partition ACT/DVE bias at the PSUM drain, so the
  drain produces the final output value.

Host staging: t is shipped twice -- as fp8e4 pairs (d-major, pre-scaled by
-2) for the DoubleRow GEMM, and as row-major bf16 for t_sq, which is one
fused DVE tensor_tensor_reduce (square + row-sum -> [128,1]) per j-chunk.
s, W ship as bf16.  s_sq comes from a ones-matmul over squared bf16 s_proj.
"""

import numpy as np
import ml_dtypes

import concourse.bacc as bacc
import concourse.bass as bass
import concourse.mybir as mybir
import concourse.tile as tile
from concourse.bass_utils import run_bass_kernel_spmd

N = 4096          # t_rep rows
M = 4096          # s_rep rows
D = 1536          # feature dim
NCORES = 8
MC = M // NCORES  # 512: output columns per core
KC = D // 128     # 12:  contraction chunks
KP = KC // 2      # 6:   fp8 DoubleRow k-pairs
NJ = N // 128     # 32:  output row chunks per core
NG = N // 512     # 8:   512-row groups
WP = D // 512     # 3:   W column pieces

FP32 = mybir.dt.float32
BF16 = mybir.dt.bfloat16
FP8 = mybir.dt.float8e4
AF = mybir.ActivationFunctionType
ALU = mybir.AluOpType
BF16NP = ml_dtypes.bfloat16
FP8NP = ml_dtypes.float8_e4m3

N_WARM = 14


def build_nc(variant="full"):
    fp8_main = variant == "full"
    # fp8: psum holds -2*cross + s_sq, drain adds t_sq bias, scale 1.
    # bf16: psum holds cross - s_sq/2, drain scales by -2 and adds t_sq.
    fold_scale = 1.0 if fp8_main else -0.5
    nc = bacc.Bacc("TRN2", target_bir_lowering=False, num_devices=NCORES)

    t8_in = nc.dram_tensor("t8", [NG, 128, KP, 2, 512], FP8,
                           kind="ExternalInput").ap()
    tr_in = nc.dram_tensor("tr", [NG, 128, 4, D], BF16,
                           kind="ExternalInput").ap()
    s_in = nc.dram_tensor("s", [128, KP, 2, MC], FP8, kind="ExternalInput").ap()
    w_in = nc.dram_tensor("w", [WP, KP, 128, 2, 512], FP8,
                          kind="ExternalInput").ap()
    b_in = nc.dram_tensor("b", [128, KC], FP32, kind="ExternalInput").ap()
    out = nc.dram_tensor("out", [NJ, 128, MC], FP32, kind="ExternalOutput").ap()

    with tile.TileContext(nc) as tc:
        with (
            tc.tile_pool(name="const", bufs=1) as const_pool,
            tc.tile_pool(name="sproj", bufs=1) as sproj_pool,
            tc.tile_pool(name="sprojf8", bufs=1) as sprojf8_pool,
            tc.tile_pool(name="small", bufs=2) as small_pool,
            tc.tile_pool(name="psum_main", bufs=4, space="PSUM") as psum_main,
        ):
            ones_col = const_pool.tile([128, 1], BF16)
            nc.vector.memset(ones_col[:], 1.0)
            ones128 = const_pool.tile([1, 128], BF16)
            nc.vector.memset(ones128[:], 1.0)
            ssq_bf = const_pool.tile([1, MC], BF16)  # fold rhs (written ph.1)

            # ---- Phase 0: HAM warmup while initial DMAs stream ----
            with (
                tc.tile_pool(name="warmp", bufs=1) as warm_pool,
                tc.tile_pool(name="psum_warm", bufs=1, space="PSUM") as pw_pool,
            ):
                warm = warm_pool.tile([128, MC], BF16, name="warm")
                nc.vector.memset(warm[:], 0.5)
                pw = pw_pool.tile([128, MC], FP32, name="pw")
                for i in range(N_WARM):
                    nc.tensor.matmul(pw[:], lhsT=warm[:, 0:128], rhs=warm[:],
                                     start=(i == 0), stop=(i == N_WARM - 1))

            # ---- Phase 1: projection s_projT + s_sq; t groups stream in ----
            sproj = []    # 12 bf16 tiles [128, MC]
            sprojf8 = []  # 6 fp8 pair tiles [128, 2, MC]
            for p in range(KP):
                spf = sprojf8_pool.tile([128, 2, MC], FP8, name=f"sprojf8_{p}")
                sprojf8.append(spf)

            with (
                tc.tile_pool(name="wts", bufs=1) as wt_pool,
                tc.tile_pool(name="srep", bufs=1) as s_pool,
                tc.tile_pool(name="bias", bufs=1) as b_pool,
                tc.tile_pool(name="sq", bufs=KC) as sq_pool,
                tc.tile_pool(name="t8sb", bufs=3) as t8_pool,
                tc.tile_pool(name="trsb", bufs=3) as tr_pool,
                tc.tile_pool(name="ttrscratch", bufs=2) as ttr_pool,
                tc.tile_pool(name="tsqb", bufs=24) as tsq_pool,
                tc.tile_pool(name="osb", bufs=16) as out_pool,
                tc.tile_pool(name="psum_ssq", bufs=1, space="PSUM") as psum_ssq_pool,
            ):
                # -- DMA issue order: s, W pieces (c-major), b, early t groups --
                s8_sb = s_pool.tile([128, KP, 2, MC], FP8, name="s8_sb")
                nc.sync.dma_start(out=s8_sb[:], in_=s_in[:, :, :, :])
                w8_sb = [wt_pool.tile([128, 2, D], FP8, name=f"w8_{p}")
                         for p in range(KP)]
                for c in range(WP):
                    for p in range(KP):
                        sl = slice(c * 512, (c + 1) * 512)
                        nc.sync.dma_start(out=w8_sb[p][:, :, sl], in_=w_in[c, p])
                b_sb = b_pool.tile([128, KC], FP32, name="b_sb")
                nc.sync.dma_start(out=b_sb[:], in_=b_in[:, :])
                t8_sb, tr_sb = {}, {}
                for g in range(3):
                    t8 = t8_pool.tile([128, KP, 2, 512], FP8, name="t8")
                    nc.sync.dma_start(out=t8[:], in_=t8_in[g])
                    t8_sb[g] = t8
                    tr = tr_pool.tile([128, 4, D], BF16, name="tr")
                    nc.sync.dma_start(out=tr[:], in_=tr_in[g])
                    tr_sb[g] = tr

                # -- GEMM1: 12 j-blocks of 6 fp8 DoubleRow k-pair matmuls --
                psum_ssq = psum_ssq_pool.tile([1, MC], FP32, name="psum_ssq")
                sq_sb = []
                for j in range(KC):
                    ps = psum_main.tile([128, MC], FP32, name="psum_p1", tag="mm")
                    for p in range(KP):
                        nc.tensor.matmul(
                            ps[:],
                            lhsT=w8_sb[p][:, :, j * 128:(j + 1) * 128],
                            rhs=s8_sb[:, p, :, :],
                            start=(p == 0),
                            stop=(p == KP - 1),
                            perf_mode=mybir.MatmulPerfMode.DoubleRow,
                        )
                    sp = sproj_pool.tile([128, MC], BF16, name=f"sproj{j}")
                    nc.scalar.activation(sp[:], ps[:], AF.Identity,
                                         bias=b_sb[:, j:j + 1], scale=1.0)
                    sproj.append(sp)
                    if fp8_main:
                        # fp8 copy for the DoubleRow GEMM (DVE reads same psum)
                        nc.vector.tensor_scalar_add(
                            sprojf8[j // 2][:, j % 2, :], ps[:], b_sb[:, j:j + 1])
                    # squared projection for s_sq
                    sq = sq_pool.tile([128, MC], BF16, name="sq")
                    nc.vector.tensor_mul(sq[:], sp[:], sp[:])
                    sq_sb.append(sq)
                    # lag the s_sq ones-matmul two j-blocks so PE never waits
                    if j >= 2:
                        nc.tensor.matmul(psum_ssq[:], lhsT=ones_col[:],
                                         rhs=sq_sb[j - 2][:],
                                         start=(j == 2), stop=False)
                for j in (KC - 2, KC - 1):
                    nc.tensor.matmul(psum_ssq[:], lhsT=ones_col[:],
                                     rhs=sq_sb[j][:],
                                     start=False, stop=(j == KC - 1))
                nc.scalar.activation(ssq_bf[:], psum_ssq[:], AF.Identity,
                                     scale=fold_scale)

                # ---- Phase 2: main fp8 GEMM over 8 groups x 4 j-chunks ----
                pending_out = []
                for g in range(NG):
                    if g + 3 < NG:
                        t8 = t8_pool.tile([128, KP, 2, 512], FP8, name="t8")
                        nc.sync.dma_start(out=t8[:], in_=t8_in[g + 3])
                        t8_sb[g + 3] = t8
                        tr = tr_pool.tile([128, 4, D], BF16, name="tr")
                        nc.sync.dma_start(out=tr[:], in_=tr_in[g + 3])
                        tr_sb[g + 3] = tr
                    t8 = t8_sb.pop(g)
                    tr = tr_sb.pop(g)
                    # t_sq -> [128, 1] drain bias.  Two chunks on ScalarE
                    # (fused Square + row-sum accum_out), two on DVE via
                    # bn_stats/bn_aggr (one pass + tiny fixup).
                    tsq = []
                    for jj in range(4):
                        tb = tsq_pool.tile([128, 1], FP32, name="tb")
                        if jj < 2:
                            scr = ttr_pool.tile([128, D], BF16, name="scr")
                            nc.scalar.activation(scr[:], tr[:, jj, :], AF.Square,
                                                 accum_out=tb[:])
                        else:
                            bn6 = tsq_pool.tile([128, 3, 6], FP32, name="bn6")
                            for cc in range(3):
                                nc.vector.bn_stats(
                                    bn6[:, cc, :],
                                    tr[:, jj, cc * 512:(cc + 1) * 512])
                            bn2 = tsq_pool.tile([128, 2], FP32, name="bn2")
                            nc.vector.bn_aggr(bn2[:], bn6[:])
                            # t_sq = D * (var + mean^2)
                            msq = tsq_pool.tile([128, 1], FP32, name="msq")
                            nc.vector.tensor_mul(msq[:], bn2[:, 0:1], bn2[:, 0:1])
                            nc.vector.tensor_add(msq[:], msq[:], bn2[:, 1:2])
                            nc.vector.tensor_scalar_mul(tb[:], msq[:], float(D))
                        tsq.append(tb)

                    # flush deferred output stores (issued well after their
                    # drain so the sync stream never waits); drain the queue
                    # harder near the end so the tail doesn't serialize
                    flush_thr = 7 if g < NG - 2 else 2
                    while len(pending_out) > flush_thr:
                        oj, oob = pending_out.pop(0)
                        nc.sync.dma_start(out=out[oj], in_=oob[:])

                    for jj in range(4):
                        j = 4 * g + jj
                        ps = psum_main.tile([128, MC], FP32, name="psum_main",
                                            tag="mm")
                        if fp8_main:
                            # stop=True on the last DoubleRow closes the sim's
                            # group tracker; the fold below still accumulates
                            # (stop is a no-op on hardware) with the group
                            # check skipped.
                            for p in range(KP):
                                nc.tensor.matmul(
                                    ps[:],
                                    lhsT=t8[:, p, :, jj * 128:(jj + 1) * 128],
                                    rhs=sprojf8[p][:],
                                    start=(p == 0),
                                    stop=(p == KP - 1),
                                    perf_mode=mybir.MatmulPerfMode.DoubleRow,
                                )
                        else:
                            for p in range(KP):
                                for c in range(2):
                                    # bf16 fallback reuses the fp8 tiles is not
                                    # possible; use row-major tr as lhsT via
                                    # sproj stream: not supported -> fp8 only
                                    raise NotImplementedError
                        # fold matmul: += s_sq (K=1, ones x ssq row)
                        nc.tensor.matmul(
                            ps[:],
                            lhsT=ones128[:],
                            rhs=ssq_bf[:],
                            start=False,
                            stop=True,
                            skip_group_check=True,
                        )
                        ob = out_pool.tile([128, MC], FP32, name="osb")
                        if jj % 2 == 0:
                            nc.scalar.activation(ob[:], ps[:], AF.Identity,
                                                 bias=tsq[jj][:], scale=1.0)
                        else:
                            nc.vector.tensor_scalar_add(ob[:], ps[:], tsq[jj][:])
                        pending_out.append((j, ob))
                for (oj, oob) in pending_out:
                    nc.sync.dma_start(out=out[oj], in_=oob[:])

    nc.compile()
    return nc


_NC_CACHE = {}


def _get_nc(variant="full"):
    if variant not in _NC_CACHE:
        _NC_CACHE[variant] = build_nc(variant)
    return _NC_CACHE[variant]


def stage_inputs(t_rep, s_rep, W, b):
    """Host-side layout staging (transpose/tile + dtype cast) -> per-core inputs."""
    t_rep = np.asarray(t_rep, dtype=np.float32)
    s_rep = np.asarray(s_rep, dtype=np.float32)
    W = np.asarray(W, dtype=np.float32)
    b = np.asarray(b, dtype=np.float32)

    # fp8 pairs, d-major, pre-scaled by -2:
    #   t8[g][p][P][c][r] = -2 * t[g*512+r, (2P+c)*128+p]
    t8 = np.ascontiguousarray(
        (-2.0 * t_rep).astype(FP8NP)
        .reshape(NG, 512, KP, 2, 128).transpose(0, 4, 2, 3, 1)
    )
    # row-major bf16: tr[g][p][jj][d] = t[g*512+jj*128+p, d]
    tr = np.ascontiguousarray(
        t_rep.astype(BF16NP).reshape(NG, 4, 128, D).transpose(0, 2, 1, 3)
    )
    # W fp8 pairs, piece-major: w8[c, P][p, cc, m] = W[c*512+m, (2P+cc)*128+p]
    wt = np.ascontiguousarray(
        W.T.astype(FP8NP).reshape(KP, 2, 128, WP, 512).transpose(3, 0, 2, 1, 4)
    )
    # b: [128, KC]; b_st[p, k] = b[k*128+p]
    b_st = np.ascontiguousarray(b.reshape(KC, 128).T)

    in_maps = []
    for c in range(NCORES):
        s_slice = s_rep[c * MC:(c + 1) * MC]  # [512, D]
        # fp8 pairs, partition-major: s8[p, P, cc, r] = s[r, (2P+cc)*128+p]
        s_st = np.ascontiguousarray(
            s_slice.astype(FP8NP).reshape(MC, KP, 2, 128).transpose(3, 1, 2, 0)
        )
        in_maps.append({"t8": t8, "tr": tr, "s": s_st, "w": wt, "b": b_st})
    return in_maps


def run_spmd(in_maps, variant="full", **kwargs):
    nc = _get_nc(variant)
    return run_bass_kernel_spmd(nc, in_maps, core_ids=list(range(NCORES)), **kwargs)


def gather_output(results):
    return np.concatenate(
        [results[c]["out"].reshape(N, MC) for c in range(NCORES)], axis=1
    )


def kernel(t_rep, s_rep, W, b):
    in_maps = stage_inputs(t_rep, s_rep, W, b)
    res = run_spmd(in_maps)
    return gather_output(res.results)


# revision 24
# speedup vs baseline: 1.9399x; 1.1236x over previous
"""Bass/Tile TRN2 kernel for retrieval-KNN MSE distance matrix.

Computes: out = ||t||^2 + ||s@W.T+b||^2 - 2 * t @ (s@W.T+b).T   [N=4096, M=4096]

Sharding (8 cores, output column-parallel, no collectives):
  core c holds s_rep rows [c*512, (c+1)*512) and computes the full-height
  output block out[:, c*512:(c+1)*512].  Per-core work:
    GEMM1: s_projT[1536, 512] = WT.T @ sT       (bf16, 12x12 k/j chunks)
    GEMM2: psum[128, 512] = (-2 t) @ s_projT    (fp8e4 DoubleRow, 6 K=256 MMs)
           + one bf16 K=1 "fold" matmul adding s_sq
  t_sq enters as the per-partition ACT/DVE bias at the PSUM drain, so the
  drain produces the final output value.

Host staging: t is shipped twice -- as fp8e4 pairs (d-major, pre-scaled by
-2) for the DoubleRow GEMM, and as row-major bf16 for t_sq, which is one
fused DVE tensor_tensor_reduce (square + row-sum -> [128,1]) per j-chunk.
s, W ship as bf16.  s_sq comes from a ones-matmul over squared bf16 s_proj.
"""

import numpy as np
import ml_dtypes

import concourse.bacc as bacc
import concourse.bass as bass
import concourse.mybir as mybir
import concourse.tile as tile
from concourse.bass_utils import run_bass_kernel_spmd

N = 4096          # t_rep rows
M = 4096          # s_rep rows
D = 1536          # feature dim
NCORES = 8
MC = M // NCORES  # 512: output columns per core
KC = D // 128     # 12:  contraction chunks
KP = KC // 2      # 6:   fp8 DoubleRow k-pairs
NJ = N // 128     # 32:  output row chunks per core
NG = N // 512     # 8:   512-row groups
WP = D // 512     # 3:   W column pieces

FP32 = mybir.dt.float32
BF16 = mybir.dt.bfloat16
FP8 = mybir.dt.float8e4
AF = mybir.ActivationFunctionType
ALU = mybir.AluOpType
BF16NP = ml_dtypes.bfloat16
FP8NP = ml_dtypes.float8_e4m3

N_WARM = 12


def build_nc(variant="full"):
    fp8_main = variant == "full"
    # fp8: psum holds -2*cross + s_sq, drain adds t_sq bias, scale 1.
    # bf16: psum holds cross - s_sq/2, drain scales by -2 and adds t_sq.
    fold_scale = 1.0 if fp8_main else -0.5
    nc = bacc.Bacc("TRN2", target_bir_lowering=False, num_devices=NCORES)

    t8_in = nc.dram_tensor("t8", [NG, 128, KP, 2, 512], FP8,
                           kind="ExternalInput").ap()
    tr_in = nc.dram_tensor("tr", [NG, 128, 4, D], FP8,
                           kind="ExternalInput").ap()
    s_in = nc.dram_tensor("s", [128, KP, 2, MC], FP8, kind="ExternalInput").ap()
    w_in = nc.dram_tensor("w", [WP, KP, 128, 2, 512], FP8,
                          kind="ExternalInput").ap()
    b_in = nc.dram_tensor("b", [128, KC], FP32, kind="ExternalInput").ap()
    out = nc.dram_tensor("out", [NJ, 128, MC], FP32, kind="ExternalOutput").ap()

    with tile.TileContext(nc) as tc:
        with (
            tc.tile_pool(name="const", bufs=1) as const_pool,
            tc.tile_pool(name="sproj", bufs=1) as sproj_pool,
            tc.tile_pool(name="sprojf8", bufs=1) as sprojf8_pool,
            tc.tile_pool(name="small", bufs=2) as small_pool,
            tc.tile_pool(name="psum_main", bufs=4, space="PSUM") as psum_main,
        ):
            ones_col = const_pool.tile([128, 1], BF16)
            nc.vector.memset(ones_col[:], 1.0)
            ones128 = const_pool.tile([1, 128], BF16)
            nc.vector.memset(ones128[:], 1.0)
            ssq_bf = const_pool.tile([1, MC], BF16)  # fold rhs (written ph.1)

            # ---- Phase 0: HAM warmup while initial DMAs stream ----
            with (
                tc.tile_pool(name="warmp", bufs=1) as warm_pool,
                tc.tile_pool(name="psum_warm", bufs=1, space="PSUM") as pw_pool,
            ):
                warm = warm_pool.tile([128, MC], BF16, name="warm")
                nc.vector.memset(warm[:], 0.5)
                pw = pw_pool.tile([128, MC], FP32, name="pw")
                for i in range(N_WARM):
                    nc.tensor.matmul(pw[:], lhsT=warm[:, 0:128], rhs=warm[:],
                                     start=(i == 0), stop=(i == N_WARM - 1))

            # ---- Phase 1: projection s_projT + s_sq; t groups stream in ----
            sproj = []    # 12 bf16 tiles [128, MC]
            sprojf8 = []  # 6 fp8 pair tiles [128, 2, MC]
            for p in range(KP):
                spf = sprojf8_pool.tile([128, 2, MC], FP8, name=f"sprojf8_{p}")
                sprojf8.append(spf)

            with (
                tc.tile_pool(name="wts", bufs=1) as wt_pool,
                tc.tile_pool(name="srep", bufs=1) as s_pool,
                tc.tile_pool(name="bias", bufs=1) as b_pool,
                tc.tile_pool(name="sq", bufs=KC) as sq_pool,
                tc.tile_pool(name="t8sb", bufs=3) as t8_pool,
                tc.tile_pool(name="trsb", bufs=3) as tr_pool,
                tc.tile_pool(name="ttrscratch", bufs=2) as ttr_pool,
                tc.tile_pool(name="tsqb", bufs=24) as tsq_pool,
                tc.tile_pool(name="osb", bufs=16) as out_pool,
                tc.tile_pool(name="psum_ssq", bufs=1, space="PSUM") as psum_ssq_pool,
            ):
                # -- DMA issue order: s, W pieces (c-major), b, early t groups --
                s8_sb = s_pool.tile([128, KP, 2, MC], FP8, name="s8_sb")
                nc.sync.dma_start(out=s8_sb[:], in_=s_in[:, :, :, :])
                w8_sb = [wt_pool.tile([128, 2, D], FP8, name=f"w8_{p}")
                         for p in range(KP)]
                for c in range(WP):
                    for p in range(KP):
                        sl = slice(c * 512, (c + 1) * 512)
                        nc.sync.dma_start(out=w8_sb[p][:, :, sl], in_=w_in[c, p])
                b_sb = b_pool.tile([128, KC], FP32, name="b_sb")
                nc.sync.dma_start(out=b_sb[:], in_=b_in[:, :])
                t8_sb, tr_sb = {}, {}
                for g in range(3):
                    t8 = t8_pool.tile([128, KP, 2, 512], FP8, name="t8")
                    nc.sync.dma_start(out=t8[:], in_=t8_in[g])
                    t8_sb[g] = t8
                    tr = tr_pool.tile([128, 4, D], FP8, name="tr")
                    nc.sync.dma_start(out=tr[:], in_=tr_in[g])
                    tr_sb[g] = tr

                # -- GEMM1: 12 j-blocks of 6 fp8 DoubleRow k-pair matmuls --
                psum_ssq = psum_ssq_pool.tile([1, MC], FP32, name="psum_ssq")
                sq_sb = []
                for j in range(KC):
                    ps = psum_main.tile([128, MC], FP32, name="psum_p1", tag="mm")
                    for p in range(KP):
                        nc.tensor.matmul(
                            ps[:],
                            lhsT=w8_sb[p][:, :, j * 128:(j + 1) * 128],
                            rhs=s8_sb[:, p, :, :],
                            start=(p == 0),
                            stop=(p == KP - 1),
                            perf_mode=mybir.MatmulPerfMode.DoubleRow,
                        )
                    sp = sproj_pool.tile([128, MC], BF16, name=f"sproj{j}")
                    nc.scalar.activation(sp[:], ps[:], AF.Identity,
                                         bias=b_sb[:, j:j + 1], scale=1.0)
                    sproj.append(sp)
                    if fp8_main:
                        # fp8 copy for the DoubleRow GEMM (DVE reads same psum)
                        nc.vector.tensor_scalar_add(
                            sprojf8[j // 2][:, j % 2, :], ps[:], b_sb[:, j:j + 1])
                    # squared projection for s_sq
                    sq = sq_pool.tile([128, MC], BF16, name="sq")
                    nc.vector.tensor_mul(sq[:], sp[:], sp[:])
                    sq_sb.append(sq)
                    # lag the s_sq ones-matmul two j-blocks so PE never waits
                    if j >= 2:
                        nc.tensor.matmul(psum_ssq[:], lhsT=ones_col[:],
                                         rhs=sq_sb[j - 2][:],
                                         start=(j == 2), stop=False)
                for j in (KC - 2, KC - 1):
                    nc.tensor.matmul(psum_ssq[:], lhsT=ones_col[:],
                                     rhs=sq_sb[j][:],
                                     start=False, stop=(j == KC - 1))
                nc.scalar.activation(ssq_bf[:], psum_ssq[:], AF.Identity,
                                     scale=fold_scale)

                # ---- Phase 2: main fp8 GEMM over 8 groups x 4 j-chunks ----
                pending_out = []
                for g in range(NG):
                    if g + 3 < NG:
                        t8 = t8_pool.tile([128, KP, 2, 512], FP8, name="t8")
                        nc.sync.dma_start(out=t8[:], in_=t8_in[g + 3])
                        t8_sb[g + 3] = t8
                        tr = tr_pool.tile([128, 4, D], FP8, name="tr")
                        nc.sync.dma_start(out=tr[:], in_=tr_in[g + 3])
                        tr_sb[g + 3] = tr
                    t8 = t8_sb.pop(g)
                    tr = tr_sb.pop(g)
                    # t_sq -> [128, 1] drain bias.  Two chunks on ScalarE
                    # (fused Square + row-sum accum_out), two on DVE via
                    # bn_stats/bn_aggr (one pass + tiny fixup).
                    tsq = []
                    for jj in range(4):
                        tb = tsq_pool.tile([128, 1], FP32, name="tb")
                        if jj < 2:
                            scr = ttr_pool.tile([128, D], BF16, name="scr")
                            nc.scalar.activation(scr[:], tr[:, jj, :], AF.Square,
                                                 accum_out=tb[:])
                        else:
                            bn6 = tsq_pool.tile([128, 3, 6], FP32, name="bn6")
                            for cc in range(3):
                                nc.vector.bn_stats(
                                    bn6[:, cc, :],
                                    tr[:, jj, cc * 512:(cc + 1) * 512])
                            bn2 = tsq_pool.tile([128, 2], FP32, name="bn2")
                            nc.vector.bn_aggr(bn2[:], bn6[:])
                            # t_sq = D * (var + mean^2)
                            msq = tsq_pool.tile([128, 1], FP32, name="msq")
                            nc.vector.tensor_mul(msq[:], bn2[:, 0:1], bn2[:, 0:1])
                            nc.vector.tensor_add(msq[:], msq[:], bn2[:, 1:2])
                            nc.vector.tensor_scalar_mul(tb[:], msq[:], float(D))
                        tsq.append(tb)

                    # flush deferred output stores (issued well after their
                    # drain so the sync stream never waits); drain the queue
                    # harder near the end so the tail doesn't serialize
                    while len(pending_out) > 2:
                        oj, oob = pending_out.pop(0)
                        nc.gpsimd.dma_start(out=out[oj], in_=oob[:])

                    for jj in range(4):
                        j = 4 * g + jj
                        ps = psum_main.tile([128, MC], FP32, name="psum_main",
                                            tag="mm")
                        if fp8_main:
                            # stop=True on the last DoubleRow closes the sim's
                            # group tracker; the fold below still accumulates
                            # (stop is a no-op on hardware) with the group
                            # check skipped.
                            for p in range(KP):
                                nc.tensor.matmul(
                                    ps[:],
                                    lhsT=t8[:, p, :, jj * 128:(jj + 1) * 128],
                                    rhs=sprojf8[p][:],
                                    start=(p == 0),
                                    stop=(p == KP - 1),
                                    perf_mode=mybir.MatmulPerfMode.DoubleRow,
                                )
                        else:
                            for p in range(KP):
                                for c in range(2):
                                    # bf16 fallback reuses the fp8 tiles is not
                                    # possible; use row-major tr as lhsT via
                                    # sproj stream: not supported -> fp8 only
                                    raise NotImplementedError
                        # fold matmul: += s_sq (K=1, ones x ssq row)
                        nc.tensor.matmul(
                            ps[:],
                            lhsT=ones128[:],
                            rhs=ssq_bf[:],
                            start=False,
                            stop=True,
                            skip_group_check=True,
                        )
                        ob = out_pool.tile([128, MC], FP32, name="osb")
                        if jj % 2 == 0:
                            nc.scalar.activation(ob[:], ps[:], AF.Identity,
                                                 bias=tsq[jj][:], scale=1.0)
                        else:
                            nc.vector.tensor_scalar_add(ob[:], ps[:], tsq[jj][:])
                        pending_out.append((j, ob))
                for (oj, oob) in pending_out:
                    nc.gpsimd.dma_start(out=out[oj], in_=oob[:])

    nc.compile()
    return nc


_NC_CACHE = {}


def _get_nc(variant="full"):
    if variant not in _NC_CACHE:
        _NC_CACHE[variant] = build_nc(variant)
    return _NC_CACHE[variant]


def stage_inputs(t_rep, s_rep, W, b):
    """Host-side layout staging (transpose/tile + dtype cast) -> per-core inputs."""
    t_rep = np.asarray(t_rep, dtype=np.float32)
    s_rep = np.asarray(s_rep, dtype=np.float32)
    W = np.asarray(W, dtype=np.float32)
    b = np.asarray(b, dtype=np.float32)

    # fp8 pairs, d-major, pre-scaled by -2:
    #   t8[g][p][P][c][r] = -2 * t[g*512+r, (2P+c)*128+p]
    t8 = np.ascontiguousarray(
        (-2.0 * t_rep).astype(FP8NP)
        .reshape(NG, 512, KP, 2, 128).transpose(0, 4, 2, 3, 1)
    )
    # row-major fp8: tr[g][p][jj][d] = t[g*512+jj*128+p, d]
    tr = np.ascontiguousarray(
        t_rep.astype(FP8NP).reshape(NG, 4, 128, D).transpose(0, 2, 1, 3)
    )
    # W fp8 pairs, piece-major: w8[c, P][p, cc, m] = W[c*512+m, (2P+cc)*128+p]
    wt = np.ascontiguousarray(
        W.T.astype(FP8NP).reshape(KP, 2, 128, WP, 512).transpose(3, 0, 2, 1, 4)
    )
    # b: [128, KC]; b_st[p, k] = b[k*128+p]
    b_st = np.ascontiguousarray(b.reshape(KC, 128).T)

    in_maps = []
    for c in range(NCORES):
        s_slice = s_rep[c * MC:(c + 1) * MC]  # [512, D]
        # fp8 pairs, partition-major: s8[p, P, cc, r] = s[r, (2P+cc)*128+p]
        s_st = np.ascontiguousarray(
            s_slice.astype(FP8NP).reshape(MC, KP, 2, 128).transpose(3, 1, 2, 0)
        )
        in_maps.append({"t8": t8, "tr": tr, "s": s_st, "w": wt, "b": b_st})
    return in_maps


def run_spmd(in_maps, variant="full", **kwargs):
    nc = _get_nc(variant)
    return run_bass_kernel_spmd(nc, in_maps, core_ids=list(range(NCORES)), **kwargs)


def gather_output(results):
    return np.concatenate(
        [results[c]["out"].reshape(N, MC) for c in range(NCORES)], axis=1
    )


def kernel(t_rep, s_rep, W, b):
    in_maps = stage_inputs(t_rep, s_rep, W, b)
    res = run_spmd(in_maps)
    return gather_output(res.results)


# revision 25
# speedup vs baseline: 1.9434x; 1.0018x over previous
"""Bass/Tile TRN2 kernel for retrieval-KNN MSE distance matrix.

Computes: out = ||t||^2 + ||s@W.T+b||^2 - 2 * t @ (s@W.T+b).T   [N=4096, M=4096]

Sharding (8 cores, output column-parallel, no collectives):
  core c holds s_rep rows [c*512, (c+1)*512) and computes the full-height
  output block out[:, c*512:(c+1)*512].  Per-core work:
    GEMM1: s_projT[1536, 512] = WT.T @ sT       (bf16, 12x12 k/j chunks)
    GEMM2: psum[128, 512] = (-2 t) @ s_projT    (fp8e4 DoubleRow, 6 K=256 MMs)
           + one bf16 K=1 "fold" matmul adding s_sq
  t_sq enters as the per-partition ACT/DVE bias at the PSUM drain, so the
  drain produces the final output value.

Host staging: t is shipped twice -- as fp8e4 pairs (d-major, pre-scaled by
-2) for the DoubleRow GEMM, and as row-major bf16 for t_sq, which is one
fused DVE tensor_tensor_reduce (square + row-sum -> [128,1]) per j-chunk.
s, W ship as bf16.  s_sq comes from a ones-matmul over squared bf16 s_proj.
"""

import numpy as np
import ml_dtypes

import concourse.bacc as bacc
import concourse.bass as bass
import concourse.mybir as mybir
import concourse.tile as tile
from concourse.bass_utils import run_bass_kernel_spmd

N = 4096          # t_rep rows
M = 4096          # s_rep rows
D = 1536          # feature dim
NCORES = 8
MC = M // NCORES  # 512: output columns per core
KC = D // 128     # 12:  contraction chunks
KP = KC // 2      # 6:   fp8 DoubleRow k-pairs
NJ = N // 128     # 32:  output row chunks per core
NG = N // 512     # 8:   512-row groups
WP = D // 512     # 3:   W column pieces

FP32 = mybir.dt.float32
BF16 = mybir.dt.bfloat16
FP8 = mybir.dt.float8e4
AF = mybir.ActivationFunctionType
ALU = mybir.AluOpType
BF16NP = ml_dtypes.bfloat16
FP8NP = ml_dtypes.float8_e4m3

N_WARM = 12


def build_nc(variant="full"):
    fp8_main = variant == "full"
    # fp8: psum holds -2*cross + s_sq, drain adds t_sq bias, scale 1.
    # bf16: psum holds cross - s_sq/2, drain scales by -2 and adds t_sq.
    fold_scale = 1.0 if fp8_main else -0.5
    nc = bacc.Bacc("TRN2", target_bir_lowering=False, num_devices=NCORES)

    t8_in = nc.dram_tensor("t8", [NG, 128, KP, 2, 512], FP8,
                           kind="ExternalInput").ap()
    tr_in = nc.dram_tensor("tr", [NG, 128, 4, D], FP8,
                           kind="ExternalInput").ap()
    s_in = nc.dram_tensor("s", [128, KP, 2, MC], FP8, kind="ExternalInput").ap()
    w_in = nc.dram_tensor("w", [WP, KP, 128, 2, 512], FP8,
                          kind="ExternalInput").ap()
    b_in = nc.dram_tensor("b", [128, KC], FP32, kind="ExternalInput").ap()
    out = nc.dram_tensor("out", [NJ, 128, MC], BF16, kind="ExternalOutput").ap()

    with tile.TileContext(nc) as tc:
        with (
            tc.tile_pool(name="const", bufs=1) as const_pool,
            tc.tile_pool(name="sproj", bufs=1) as sproj_pool,
            tc.tile_pool(name="sprojf8", bufs=1) as sprojf8_pool,
            tc.tile_pool(name="small", bufs=2) as small_pool,
            tc.tile_pool(name="psum_main", bufs=4, space="PSUM") as psum_main,
        ):
            ones_col = const_pool.tile([128, 1], BF16)
            nc.vector.memset(ones_col[:], 1.0)
            ones128 = const_pool.tile([1, 128], BF16)
            nc.vector.memset(ones128[:], 1.0)
            ssq_bf = const_pool.tile([1, MC], BF16)  # fold rhs (written ph.1)

            # ---- Phase 0: HAM warmup while initial DMAs stream ----
            with (
                tc.tile_pool(name="warmp", bufs=1) as warm_pool,
                tc.tile_pool(name="psum_warm", bufs=1, space="PSUM") as pw_pool,
            ):
                warm = warm_pool.tile([128, MC], BF16, name="warm")
                nc.vector.memset(warm[:], 0.5)
                pw = pw_pool.tile([128, MC], FP32, name="pw")
                for i in range(N_WARM):
                    nc.tensor.matmul(pw[:], lhsT=warm[:, 0:128], rhs=warm[:],
                                     start=(i == 0), stop=(i == N_WARM - 1))

            # ---- Phase 1: projection s_projT + s_sq; t groups stream in ----
            sproj = []    # 12 bf16 tiles [128, MC]
            sprojf8 = []  # 6 fp8 pair tiles [128, 2, MC]
            for p in range(KP):
                spf = sprojf8_pool.tile([128, 2, MC], FP8, name=f"sprojf8_{p}")
                sprojf8.append(spf)

            with (
                tc.tile_pool(name="wts", bufs=1) as wt_pool,
                tc.tile_pool(name="srep", bufs=1) as s_pool,
                tc.tile_pool(name="bias", bufs=1) as b_pool,
                tc.tile_pool(name="sq", bufs=KC) as sq_pool,
                tc.tile_pool(name="t8sb", bufs=3) as t8_pool,
                tc.tile_pool(name="trsb", bufs=3) as tr_pool,
                tc.tile_pool(name="ttrscratch", bufs=2) as ttr_pool,
                tc.tile_pool(name="tsqb", bufs=24) as tsq_pool,
                tc.tile_pool(name="osb", bufs=16) as out_pool,
                tc.tile_pool(name="psum_ssq", bufs=1, space="PSUM") as psum_ssq_pool,
            ):
                # -- DMA issue order: s, W pieces (c-major), b, early t groups --
                s8_sb = s_pool.tile([128, KP, 2, MC], FP8, name="s8_sb")
                nc.sync.dma_start(out=s8_sb[:], in_=s_in[:, :, :, :])
                w8_sb = [wt_pool.tile([128, 2, D], FP8, name=f"w8_{p}")
                         for p in range(KP)]
                for c in range(WP):
                    for p in range(KP):
                        sl = slice(c * 512, (c + 1) * 512)
                        nc.sync.dma_start(out=w8_sb[p][:, :, sl], in_=w_in[c, p])
                b_sb = b_pool.tile([128, KC], FP32, name="b_sb")
                nc.sync.dma_start(out=b_sb[:], in_=b_in[:, :])
                t8_sb, tr_sb = {}, {}
                for g in range(3):
                    t8 = t8_pool.tile([128, KP, 2, 512], FP8, name="t8")
                    nc.sync.dma_start(out=t8[:], in_=t8_in[g])
                    t8_sb[g] = t8
                    tr = tr_pool.tile([128, 4, D], FP8, name="tr")
                    nc.sync.dma_start(out=tr[:], in_=tr_in[g])
                    tr_sb[g] = tr

                # -- GEMM1: 12 j-blocks of 6 fp8 DoubleRow k-pair matmuls --
                psum_ssq = psum_ssq_pool.tile([1, MC], FP32, name="psum_ssq")
                sq_sb = []
                for j in range(KC):
                    ps = psum_main.tile([128, MC], FP32, name="psum_p1", tag="mm")
                    for p in range(KP):
                        nc.tensor.matmul(
                            ps[:],
                            lhsT=w8_sb[p][:, :, j * 128:(j + 1) * 128],
                            rhs=s8_sb[:, p, :, :],
                            start=(p == 0),
                            stop=(p == KP - 1),
                            perf_mode=mybir.MatmulPerfMode.DoubleRow,
                        )
                    sp = sproj_pool.tile([128, MC], BF16, name=f"sproj{j}")
                    nc.scalar.activation(sp[:], ps[:], AF.Identity,
                                         bias=b_sb[:, j:j + 1], scale=1.0)
                    sproj.append(sp)
                    if fp8_main:
                        # fp8 copy for the DoubleRow GEMM (DVE reads same psum)
                        nc.vector.tensor_scalar_add(
                            sprojf8[j // 2][:, j % 2, :], ps[:], b_sb[:, j:j + 1])
                    # squared projection for s_sq
                    sq = sq_pool.tile([128, MC], BF16, name="sq")
                    nc.vector.tensor_mul(sq[:], sp[:], sp[:])
                    sq_sb.append(sq)
                    # lag the s_sq ones-matmul two j-blocks so PE never waits
                    if j >= 2:
                        nc.tensor.matmul(psum_ssq[:], lhsT=ones_col[:],
                                         rhs=sq_sb[j - 2][:],
                                         start=(j == 2), stop=False)
                for j in (KC - 2, KC - 1):
                    nc.tensor.matmul(psum_ssq[:], lhsT=ones_col[:],
                                     rhs=sq_sb[j][:],
                                     start=False, stop=(j == KC - 1))
                nc.scalar.activation(ssq_bf[:], psum_ssq[:], AF.Identity,
                                     scale=fold_scale)

                # ---- Phase 2: main fp8 GEMM over 8 groups x 4 j-chunks ----
                pending_out = []
                for g in range(NG):
                    if g + 3 < NG:
                        t8 = t8_pool.tile([128, KP, 2, 512], FP8, name="t8")
                        nc.sync.dma_start(out=t8[:], in_=t8_in[g + 3])
                        t8_sb[g + 3] = t8
                        tr = tr_pool.tile([128, 4, D], FP8, name="tr")
                        nc.sync.dma_start(out=tr[:], in_=tr_in[g + 3])
                        tr_sb[g + 3] = tr
                    t8 = t8_sb.pop(g)
                    tr = tr_sb.pop(g)
                    # t_sq -> [128, 1] drain bias.  Two chunks on ScalarE
                    # (fused Square + row-sum accum_out), two on DVE via
                    # bn_stats/bn_aggr (one pass + tiny fixup).
                    tsq = []
                    for jj in range(4):
                        tb = tsq_pool.tile([128, 1], FP32, name="tb")
                        if jj < 2:
                            scr = ttr_pool.tile([128, D], BF16, name="scr")
                            nc.scalar.activation(scr[:], tr[:, jj, :], AF.Square,
                                                 accum_out=tb[:])
                        else:
                            bn6 = tsq_pool.tile([128, 3, 6], FP32, name="bn6")
                            for cc in range(3):
                                nc.vector.bn_stats(
                                    bn6[:, cc, :],
                                    tr[:, jj, cc * 512:(cc + 1) * 512])
                            bn2 = tsq_pool.tile([128, 2], FP32, name="bn2")
                            nc.vector.bn_aggr(bn2[:], bn6[:])
                            # t_sq = D * (var + mean^2)
                            msq = tsq_pool.tile([128, 1], FP32, name="msq")
                            nc.vector.tensor_mul(msq[:], bn2[:, 0:1], bn2[:, 0:1])
                            nc.vector.tensor_add(msq[:], msq[:], bn2[:, 1:2])
                            nc.vector.tensor_scalar_mul(tb[:], msq[:], float(D))
                        tsq.append(tb)

                    # flush deferred output stores (issued well after their
                    # drain so the sync stream never waits); drain the queue
                    # harder near the end so the tail doesn't serialize
                    while len(pending_out) > 1:
                        oj, oob = pending_out.pop(0)
                        nc.gpsimd.dma_start(out=out[oj], in_=oob[:])

                    for jj in range(4):
                        j = 4 * g + jj
                        ps = psum_main.tile([128, MC], FP32, name="psum_main",
                                            tag="mm")
                        if fp8_main:
                            # stop=True on the last DoubleRow closes the sim's
                            # group tracker; the fold below still accumulates
                            # (stop is a no-op on hardware) with the group
                            # check skipped.
                            for p in range(KP):
                                nc.tensor.matmul(
                                    ps[:],
                                    lhsT=t8[:, p, :, jj * 128:(jj + 1) * 128],
                                    rhs=sprojf8[p][:],
                                    start=(p == 0),
                                    stop=(p == KP - 1),
                                    perf_mode=mybir.MatmulPerfMode.DoubleRow,
                                )
                        else:
                            for p in range(KP):
                                for c in range(2):
                                    # bf16 fallback reuses the fp8 tiles is not
                                    # possible; use row-major tr as lhsT via
                                    # sproj stream: not supported -> fp8 only
                                    raise NotImplementedError
                        # fold matmul: += s_sq (K=1, ones x ssq row)
                        nc.tensor.matmul(
                            ps[:],
                            lhsT=ones128[:],
                            rhs=ssq_bf[:],
                            start=False,
                            stop=True,
                            skip_group_check=True,
                        )
                        ob = out_pool.tile([128, MC], BF16, name="osb")
                        if jj % 2 == 0:
                            nc.scalar.activation(ob[:], ps[:], AF.Identity,
                                                 bias=tsq[jj][:], scale=1.0)
                        else:
                            nc.vector.tensor_scalar_add(ob[:], ps[:], tsq[jj][:])
                        pending_out.append((j, ob))
                for (oj, oob) in pending_out:
                    nc.gpsimd.dma_start(out=out[oj], in_=oob[:])

    nc.compile()
    return nc


_NC_CACHE = {}


def _get_nc(variant="full"):
    if variant not in _NC_CACHE:
        _NC_CACHE[variant] = build_nc(variant)
    return _NC_CACHE[variant]


def stage_inputs(t_rep, s_rep, W, b):
    """Host-side layout staging (transpose/tile + dtype cast) -> per-core inputs."""
    t_rep = np.asarray(t_rep, dtype=np.float32)
    s_rep = np.asarray(s_rep, dtype=np.float32)
    W = np.asarray(W, dtype=np.float32)
    b = np.asarray(b, dtype=np.float32)

    # fp8 pairs, d-major, pre-scaled by -2:
    #   t8[g][p][P][c][r] = -2 * t[g*512+r, (2P+c)*128+p]
    t8 = np.ascontiguousarray(
        (-2.0 * t_rep).astype(FP8NP)
        .reshape(NG, 512, KP, 2, 128).transpose(0, 4, 2, 3, 1)
    )
    # row-major fp8: tr[g][p][jj][d] = t[g*512+jj*128+p, d]
    tr = np.ascontiguousarray(
        t_rep.astype(FP8NP).reshape(NG, 4, 128, D).transpose(0, 2, 1, 3)
    )
    # W fp8 pairs, piece-major: w8[c, P][p, cc, m] = W[c*512+m, (2P+cc)*128+p]
    wt = np.ascontiguousarray(
        W.T.astype(FP8NP).reshape(KP, 2, 128, WP, 512).transpose(3, 0, 2, 1, 4)
    )
    # b: [128, KC]; b_st[p, k] = b[k*128+p]
    b_st = np.ascontiguousarray(b.reshape(KC, 128).T)

    in_maps = []
    for c in range(NCORES):
        s_slice = s_rep[c * MC:(c + 1) * MC]  # [512, D]
        # fp8 pairs, partition-major: s8[p, P, cc, r] = s[r, (2P+cc)*128+p]
        s_st = np.ascontiguousarray(
            s_slice.astype(FP8NP).reshape(MC, KP, 2, 128).transpose(3, 1, 2, 0)
        )
        in_maps.append({"t8": t8, "tr": tr, "s": s_st, "w": wt, "b": b_st})
    return in_maps


def run_spmd(in_maps, variant="full", **kwargs):
    nc = _get_nc(variant)
    return run_bass_kernel_spmd(nc, in_maps, core_ids=list(range(NCORES)), **kwargs)


def gather_output(results):
    return np.concatenate(
        [results[c]["out"].reshape(N, MC).astype(np.float32)
         for c in range(NCORES)], axis=1
    )


def kernel(t_rep, s_rep, W, b):
    in_maps = stage_inputs(t_rep, s_rep, W, b)
    res = run_spmd(in_maps)
    return gather_output(res.results)
